# revision 1
# baseline (speedup 1.0000x reference)
"""GP marginal log-likelihood kernel for Trainium2 (Bass/Tile).

Computes -0.5 * y^T A^-1 y - 0.5 * logdet(A) for A = K + sigma^2 I where
K is the RBF covariance on the integer grid 0..T-1 (T=8192).

A is symmetric positive-definite *Toeplitz* and effectively *banded*
(entries vanish below f32 eps for |i-j| > 255 at lengthscale 32), and is
well conditioned: eig(A) in [sigma^2, sigma^2 + v*sum_d k(d)] (~[1, 81.2]).
This kernel exploits that structure instead of doing a dense 8192^3
factorization:

  * quad = y^T A^-1 y: x = p(A) y where p is a least-squares-optimal
    polynomial fitted (on the host, from the hyperparameters alone) to the
    *known* model spectrum of A -- the symbol samples f(2 pi j / T) -- and
    applied on device in the numerically stable Chebyshev basis:
        v_{m+1} = (2 As) v_m - v_{m-1},   x = sum_m gamma_m v_m,
    where each (2 As) v is a block-pentadiagonal matvec: 5 tensor-engine
    matmuls with 128x128 stationary band blocks.  The second-order
    functional quad = x^T (2y - A x) makes the final error quadratic in
    the solver error (~1e-5 relative at degree 18).
  * logdet via the strong Szego limit theorem:
        logdet A = T*c_0 + sum_{k>=1} k*c_k^2,   c_k = Fourier coeffs of
    log f(theta), f = the symbol of A.  For an analytic positive symbol the
    remainder decays like exp(-2*beta*T); at T=8192 it is far below f32 eps
    (verified numerically: < 1e-11 in f64, < 5e-6 in f32).  f is evaluated
    on device in closed (Poisson-summation) form with two Exps per grid
    point; the cosine/DCT matrix is generated on device (outer-product
    matmul + exact 2^23 range reduction + Sin activation).

Everything data-dependent runs on the device.  The host only computes the
iteration coefficient schedule and a handful of scalar parameters from the
scalar hyperparameters (sigma^2, lengthscale, variance); the final scalar
is assembled on core 0 and DMA'd out.  All 8 cores run the same program on
replicated inputs (the answer is a single scalar; core 0's result is
gathered).
"""

import math

import numpy as np

T = 8192
P = 128  # partitions
NBLK = T // P  # 64 column blocks
NPAD = 2  # zero pad columns on each side of the padded vec tiles
BW = 255  # band half-width kept in the 5 block matrices
N_GRID = 512  # Szego quadrature grid size (half-grid 0..256 used)
NJ = N_GRID // 2 + 1  # 257 half-grid points
K_DCT = 256  # highest Fourier coefficient kept (c_k ~ e^{-0.031k})
KC = K_DCT + 1  # DCT output columns incl. k=0
N_JTILES = 3  # ceil(257/128): 2 full partition tiles + 1 single-row
N_DEG = 18  # polynomial degree bound for the solve (17 matvecs)
MAGIC = 8388608.0  # 2^23: x + MAGIC - MAGIC == round-to-nearest(x) in f32

_prog_cache = {}


def _ls_poly(sig2, ell, var, n_deg):
    """Host-side iteration schedule: LS-optimal solve polynomial.

    Fits p(lam) = sum_m gamma_m T_m(scaled lam) minimizing
    sum_j (1 - lam_j p(lam_j))^2 / lam_j over the model spectrum
    lam_j = f(2 pi j / T) (symbol samples, the asymptotic eigenvalue
    distribution of A).  Returns (gamma, lo, hi).  Cost: a small lstsq on
    hyperparameter-derived data only -- part of the schedule, like
    Chebyshev coefficients.
    """
    th = np.linspace(0.0, np.pi, T // 2 + 1)
    lam = sig2 + var * ell * math.sqrt(2.0 * math.pi) * (
        np.exp(-((ell * th) ** 2) / 2.0)
        + np.exp(-((ell * (th - 2 * math.pi)) ** 2) / 2.0)
    )
    lo, hi = float(lam.min()), float(lam.max())
    xs = (2.0 * lam - (hi + lo)) / (hi - lo)
    V = np.zeros((lam.size, n_deg))
    V[:, 0] = 1.0
    if n_deg > 1:
        V[:, 1] = xs
    for m in range(2, n_deg):
        V[:, m] = 2.0 * xs * V[:, m - 1] - V[:, m - 2]
    w = 1.0 / lam
    Aw = V * (lam * np.sqrt(w))[:, None]
    b = np.sqrt(w)
    g, *_ = np.linalg.lstsq(Aw, b, rcond=None)
    return g, lo, hi


def _build(sig2, ell, var, n_deg, debug=False, n_copies=1, loop_n=0):
    """Emit the full program into a fresh Bacc instance and return it."""
    import concourse.mybir as mybir
    import concourse.tile as tile
    from concourse import bacc
    from concourse.masks import make_identity

    f32 = mybir.dt.float32
    i32 = mybir.dt.int32
    AF = mybir.ActivationFunctionType
    OP = mybir.AluOpType

    gam, lam_lo, lam_hi = _ls_poly(sig2, ell, var, n_deg)
    # 2*As = sc2*A + sh2*I
    sc2 = 4.0 / (lam_hi - lam_lo)
    sh2 = -2.0 * (lam_hi + lam_lo) / (lam_hi - lam_lo)

    nc = bacc.Bacc("TRN2", target_bir_lowering=False, debug=False)
    y_dram = nc.dram_tensor("y", [T], f32, kind="ExternalInput")
    # params (rows replicated so any slice works as a per-partition scalar):
    # 0: -1/(2 l^2)   1: -v      2: -sigma^2  3: sigma^2
    # 4: v*l*sqrt(2pi)  5: -l^2/2  6,7: spare
    par_dram = nc.dram_tensor("par", [P, 8], f32, kind="ExternalInput")
    out_dram = nc.dram_tensor("out", [1, n_copies], f32, kind="ExternalOutput")
    if debug:
        dbg_c = nc.dram_tensor("dbg_c", [1, KC], f32, kind="ExternalOutput")
        dbg_x = nc.dram_tensor("dbg_x", [P, NBLK], f32, kind="ExternalOutput")
        dbg_ql = nc.dram_tensor("dbg_ql", [1, 4], f32, kind="ExternalOutput")

    with tile.TileContext(nc) as tc:
        with (
            tc.tile_pool(name="const", bufs=1) as cpool,
            tc.tile_pool(name="work", bufs=1) as wpool,
            tc.tile_pool(name="dct", bufs=2) as dpool,
            tc.tile_pool(name="ps", bufs=1, space="PSUM") as ppool,
            tc.tile_pool(name="psdct", bufs=2, space="PSUM") as pdpool,
        ):
            def emit(ci):
                _emit_one(
                    nc, tc, cpool, wpool, dpool, ppool, pdpool,
                    mybir, make_identity,
                    y_dram, par_dram, out_dram,
                    dbg_c if debug and ci == 0 else None,
                    dbg_x if debug and ci == 0 else None,
                    dbg_ql if debug and ci == 0 else None,
                    gam, sc2, sh2, n_deg, ci,
                )

            if loop_n:
                with tc.For_i(0, loop_n, 1):
                    emit(0)
            else:
                for ci in range(n_copies):
                    emit(ci)

    nc.compile()
    return nc


def _emit_one(
    nc, tc, cpool, wpool, dpool, ppool, pdpool, mybir, make_identity,
    y_dram, par_dram, out_dram, dbg_c, dbg_x, dbg_ql,
    gam, sc2, sh2, n_deg, ci,
):
    from concourse.tile_rust import add_dep_helper

    f32 = mybir.dt.float32
    i32 = mybir.dt.int32
    AF = mybir.ActivationFunctionType
    OP = mybir.AluOpType

    par = cpool.tile([P, 8], f32, tag=f"par{ci}")
    nc.sync.dma_start(par[:], par_dram[:])

    ident = cpool.tile([P, P], f32, tag=f"id{ci}")
    make_identity(nc, ident[:])
    bneg2pi = cpool.tile([P, 1], f32, tag=f"bneg2pi{ci}")
    nc.vector.memset(bneg2pi[:], -2.0 * math.pi)

    # contiguous row-major load, then PE transpose into the block layout
    # ysb[r, b] = y[b*128 + r]  (a 4B-strided DMA would be descriptor-bound)
    yrow = cpool.tile([NBLK, P], f32, tag=f"yrow{ci}")
    nc.sync.dma_start(yrow[:], y_dram.rearrange("(b r) -> b r", b=NBLK))
    ysb_ps = ppool.tile([P, NBLK], f32, tag="ysb_ps")
    nc.tensor.transpose(ysb_ps[:], yrow[:], ident[:NBLK, :NBLK])
    ysb = cpool.tile([P, NBLK], f32, tag=f"ysb{ci}")
    nc.vector.tensor_copy(ysb[:], ysb_ps[:])

    # ---------------- band block matrices ----------------
    # NS[c, m, r] = -(v*exp(-(128(m-2)+c-r)^2/(2 l^2)) + sig2*[d==0])
    # NS2 = -sc2*NS + sh2*I  (the 2*As operator blocks)
    dmat_i = cpool.tile([P, 5, P], i32, tag=f"dmi{ci}")
    nc.gpsimd.iota(
        dmat_i[:], pattern=[[P, 5], [-1, P]], base=-2 * P, channel_multiplier=1
    )
    dmat = cpool.tile([P, 5, P], f32, tag=f"dm{ci}")
    nc.gpsimd.tensor_copy(dmat[:], dmat_i[:])
    nc.scalar.activation(dmat[:], dmat[:], AF.Square)
    nc.scalar.activation(dmat[:], dmat[:], AF.Exp, scale=par[:, 0:1])
    NS = cpool.tile([P, 5, P], f32, tag=f"NS{ci}")
    nc.vector.tensor_scalar(NS[:], dmat[:], par[:, 1:2], None, op0=OP.mult)
    nc.vector.scalar_tensor_tensor(
        NS[:, 2, :],
        in0=ident[:],
        scalar=par[:, 2:3],
        in1=NS[:, 2, :],
        op0=OP.mult,
        op1=OP.add,
    )
    NS2 = cpool.tile([P, 5, P], f32, tag=f"NS2{ci}")
    nc.gpsimd.tensor_scalar(NS2[:], NS[:], float(-sc2), None, op0=OP.mult)
    nc.vector.scalar_tensor_tensor(
        NS2[:, 2, :],
        in0=ident[:],
        scalar=float(sh2),
        in1=NS2[:, 2, :],
        op0=OP.mult,
        op1=OP.add,
    )

    # ---------------- Szego logdet (phase 1) ----------------
    jmat_i = cpool.tile([P, N_JTILES], i32, tag=f"jmi{ci}")
    nc.gpsimd.iota(
        jmat_i[:], pattern=[[P, N_JTILES]], base=0, channel_multiplier=1
    )
    jmat = cpool.tile([P, N_JTILES], f32, tag=f"jm{ci}")
    nc.vector.tensor_copy(jmat[:], jmat_i[:])
    th2 = wpool.tile([P, N_JTILES], f32, tag=f"th2{ci}")
    nc.scalar.activation(th2[:], jmat[:], AF.Square, scale=2.0 * math.pi / N_GRID)
    e1 = wpool.tile([P, N_JTILES], f32, tag=f"e1{ci}")
    nc.scalar.activation(e1[:], th2[:], AF.Exp, scale=par[:, 5:6])
    th2b = wpool.tile([P, N_JTILES], f32, tag=f"th2b{ci}")
    nc.scalar.activation(
        th2b[:],
        jmat[:],
        AF.Square,
        scale=2.0 * math.pi / N_GRID,
        bias=bneg2pi[:],
    )
    e2 = wpool.tile([P, N_JTILES], f32, tag=f"e2{ci}")
    nc.scalar.activation(e2[:], th2b[:], AF.Exp, scale=par[:, 5:6])
    fsym = wpool.tile([P, N_JTILES], f32, tag=f"fsym{ci}")
    nc.vector.tensor_tensor(fsym[:], e1[:], e2[:], op=OP.add)
    nc.vector.tensor_scalar(
        fsym[:], fsym[:], par[:, 4:5], par[:, 3:4], op0=OP.mult, op1=OP.add
    )
    gl = wpool.tile([P, N_JTILES], f32, tag=f"gl{ci}")
    nc.scalar.activation(gl[:], fsym[:], AF.Ln)
    wq = cpool.tile([P, N_JTILES], f32, tag=f"wq{ci}")
    nc.gpsimd.memset(wq[:], 2.0 / N_GRID)
    nc.gpsimd.memset(wq[:, N_JTILES - 1 : N_JTILES], 0.0)
    nc.gpsimd.memset(wq[0:1, 0:1], 1.0 / N_GRID)
    nc.gpsimd.memset(wq[0:1, N_JTILES - 1 : N_JTILES], 1.0 / N_GRID)
    nc.vector.tensor_tensor(gl[:], gl[:], wq[:], op=OP.mult)

    # DCT: c[k] = sum_j g~[j] cos(2 pi j k / N)
    kvec_i = cpool.tile([1, KC], i32, tag=f"kvi{ci}")
    nc.gpsimd.iota(kvec_i[:], pattern=[[1, KC]], base=0, channel_multiplier=0)
    kvec = cpool.tile([1, KC], f32, tag=f"kv{ci}")
    nc.vector.tensor_copy(kvec[:], kvec_i[:])
    kdivn = cpool.tile([1, KC], f32, tag=f"kdn{ci}")
    nc.vector.tensor_scalar(kdivn[:], kvec[:], 1.0 / N_GRID, None, op0=OP.mult)

    c_ps = ppool.tile([1, KC], f32, tag="c_ps")
    for t in range(N_JTILES):
        rows = P if t < N_JTILES - 1 else 1
        jv_i = dpool.tile([1, P], i32, tag="jv_i")
        nc.gpsimd.iota(
            jv_i[:1, :rows], pattern=[[1, rows]], base=t * P, channel_multiplier=0
        )
        jv = dpool.tile([1, P], f32, tag="jv")
        nc.vector.tensor_copy(jv[:1, :rows], jv_i[:1, :rows])
        tau_ps = pdpool.tile([P, KC], f32, tag="tau_ps")
        nc.tensor.matmul(
            tau_ps[:rows, :],
            jv[:1, :rows],
            kdivn[:],
            start=True,
            stop=True,
            skip_group_check=True,
        )
        # a1 = tau + 0.25; R = round(a1) via +-2^23 (ACT, rne adds);
        # psi = a1 - R in [-0.5, 0.5];  cos(2 pi tau) = Sin(2 pi psi)
        a1 = dpool.tile([P, KC], f32, tag="a1")
        nc.vector.tensor_scalar(
            a1[:rows, :], tau_ps[:rows, :], 0.25, None, op0=OP.add
        )
        rnd0 = dpool.tile([P, KC], f32, tag="rnd0")
        nc.scalar.activation(rnd0[:rows, :], a1[:rows, :], AF.Copy, bias=MAGIC)
        nc.scalar.activation(rnd0[:rows, :], rnd0[:rows, :], AF.Copy, bias=-MAGIC)
        psi = dpool.tile([P, KC], f32, tag="psi")
        nc.vector.scalar_tensor_tensor(
            psi[:rows, :],
            in0=rnd0[:rows, :],
            scalar=-1.0,
            in1=a1[:rows, :],
            op0=OP.mult,
            op1=OP.add,
        )
        cmat = dpool.tile([P, KC], f32, tag="cmat")
        nc.scalar.activation(
            cmat[:rows, :], psi[:rows, :], AF.Sin, scale=2.0 * math.pi
        )
        nc.tensor.matmul(
            c_ps[:],
            gl[:rows, t : t + 1],
            cmat[:rows, :],
            start=(t == 0),
            stop=(t == N_JTILES - 1),
            skip_group_check=True,
        )

    csb = wpool.tile([1, KC], f32, tag=f"csb{ci}")
    nc.vector.tensor_copy(csb[:], c_ps[:])
    ck2 = wpool.tile([1, KC], f32, tag=f"ck2{ci}")
    nc.vector.tensor_tensor(ck2[:], csb[:], csb[:], op=OP.mult)
    nc.vector.tensor_tensor(ck2[:], ck2[:], kvec[:], op=OP.mult)
    s2 = wpool.tile([1, 1], f32, tag=f"s2{ci}")
    nc.vector.tensor_reduce(s2[:], ck2[:], axis=mybir.AxisListType.X, op=OP.add)
    # logdet = T*c0 + s2
    ld = wpool.tile([1, 1], f32, tag=f"ld{ci}")
    ld_op = nc.vector.scalar_tensor_tensor(
        ld[:], in0=csb[:, 0:1], scalar=float(T), in1=s2[:], op0=OP.mult, op1=OP.add
    )

    # ---------------- polynomial solve (phase 2) ----------------
    va = wpool.tile([P, NBLK + 2 * NPAD], f32, tag=f"va{ci}")
    vb = wpool.tile([P, NBLK + 2 * NPAD], f32, tag=f"vb{ci}")
    xs = wpool.tile([P, NBLK + 2 * NPAD], f32, tag=f"xs{ci}")
    gate_ops = [
        nc.vector.memset(va[:], 0.0),
        nc.vector.memset(vb[:], 0.0),
        nc.vector.memset(xs[:], 0.0),
    ]
    W_ps = ppool.tile([P, NBLK], f32, tag="W_ps")

    def matvec(dst_ps, src, mats):
        for m in range(5):
            off = m - 2
            nc.tensor.matmul(
                dst_ps[:],
                mats[:, m, :],
                src[:, NPAD + off : NPAD + off + NBLK],
                start=(m == 0),
                stop=(m == 4),
                skip_group_check=True,
            )

    # v0 = y; x = gamma_0 * y
    gate_ops.append(nc.vector.tensor_copy(va[:, NPAD : NPAD + NBLK], ysb[:]))
    gate_ops.append(
        nc.vector.tensor_scalar(
            xs[:, NPAD : NPAD + NBLK], ysb[:], float(gam[0]), None, op0=OP.mult
        )
    )
    # phase separation: the szego path owns DVE/ACT until ld is done;
    # interleaving its big DVE ops into the solve's latency-critical
    # PE->DVE->PE loop was measured to cost ~40 us.
    for op in gate_ops:
        add_dep_helper(op.ins, ld_op.ins, sync=True, reason="phase-separation")

    # v1 = As y = 0.5 * (2As) v0
    matvec(W_ps, va, NS2)
    nc.vector.tensor_scalar(
        vb[:, NPAD : NPAD + NBLK], W_ps[:], 0.5, None, op0=OP.mult
    )
    nc.vector.scalar_tensor_tensor(
        xs[:, NPAD : NPAD + NBLK],
        in0=vb[:, NPAD : NPAD + NBLK],
        scalar=float(gam[1]),
        in1=xs[:, NPAD : NPAD + NBLK],
        op0=OP.mult,
        op1=OP.add,
    )

    vold, vcur = va, vb
    for m in range(2, n_deg):
        matvec(W_ps, vcur, NS2)
        # v_new = W - v_old   (into v_old's buffer)
        nc.vector.scalar_tensor_tensor(
            vold[:, NPAD : NPAD + NBLK],
            in0=W_ps[:],
            scalar=1.0,
            in1=vold[:, NPAD : NPAD + NBLK],
            op0=OP.mult,
            op1=OP.subtract,
        )
        vold, vcur = vcur, vold
        # x += gamma_m * v_new  (off the critical path)
        nc.vector.scalar_tensor_tensor(
            xs[:, NPAD : NPAD + NBLK],
            in0=vcur[:, NPAD : NPAD + NBLK],
            scalar=float(gam[m]),
            in1=xs[:, NPAD : NPAD + NBLK],
            op0=OP.mult,
            op1=OP.add,
        )

    # quad = x^T (2y - A x)
    mv_ps = ppool.tile([P, NBLK], f32, tag="mv_ps")
    matvec(mv_ps, xs, NS)  # mv = -A x
    y2 = wpool.tile([P, NBLK], f32, tag=f"y2{ci}")
    nc.vector.tensor_scalar(y2[:], ysb[:], 2.0, None, op0=OP.mult)
    g2 = wpool.tile([P, NBLK], f32, tag=f"g2{ci}")
    nc.vector.scalar_tensor_tensor(
        g2[:], in0=mv_ps[:], scalar=1.0, in1=y2[:], op0=OP.mult, op1=OP.add
    )
    tq = wpool.tile([P, NBLK], f32, tag=f"tq{ci}")
    nc.vector.tensor_tensor(tq[:], xs[:, NPAD : NPAD + NBLK], g2[:], op=OP.mult)
    tred = wpool.tile([P, 1], f32, tag=f"tred{ci}")
    nc.vector.tensor_reduce(tred[:], tq[:], axis=mybir.AxisListType.X, op=OP.add)
    ones = cpool.tile([P, 1], f32, tag=f"ones{ci}")
    nc.vector.memset(ones[:], 1.0)
    quad_ps = ppool.tile([1, 1], f32, tag="quad_ps")
    nc.tensor.matmul(
        quad_ps[:], tred[:], ones[:], start=True, stop=True, skip_group_check=True
    )

    # out = -0.5*(quad + logdet)
    fin = wpool.tile([1, 1], f32, tag=f"fin{ci}")
    nc.vector.scalar_tensor_tensor(
        fin[:], in0=quad_ps[:], scalar=1.0, in1=ld[:], op0=OP.mult, op1=OP.add
    )
    nc.vector.tensor_scalar(fin[:], fin[:], -0.5, None, op0=OP.mult)
    nc.sync.dma_start(out_dram[:, ci : ci + 1], fin[:])

    if dbg_c is not None:
        nc.sync.dma_start(dbg_c[:], csb[:])
        nc.sync.dma_start(dbg_x[:], xs[:, NPAD : NPAD + NBLK])
        dq = wpool.tile([1, 4], f32, tag="dq")
        nc.vector.tensor_copy(dq[:, 0:1], quad_ps[:])
        nc.vector.tensor_copy(dq[:, 1:2], ld[:])
        nc.vector.tensor_copy(dq[:, 2:3], s2[:])
        nc.vector.tensor_copy(dq[:, 3:4], csb[:, 0:1])
        nc.sync.dma_start(dbg_ql[:], dq[:])


def _params_array(sig2, ell, var):
    row = np.array(
        [
            -1.0 / (2.0 * ell * ell),
            -var,
            -sig2,
            sig2,
            var * ell * math.sqrt(2.0 * math.pi),
            -(ell * ell) / 2.0,
            0.0,
            0.0,
        ],
        dtype=np.float32,
    )
    return np.tile(row[None, :], (P, 1))


def get_program(sig2, ell, var, n_deg=N_DEG, debug=False, n_copies=1, loop_n=0):
    key = (float(sig2), float(ell), float(var), int(n_deg), bool(debug), n_copies,
           loop_n)
    if key not in _prog_cache:
        _prog_cache[key] = _build(
            *key[:4], debug=key[4], n_copies=key[5], loop_n=key[6]
        )
    return _prog_cache[key]


def kernel(y, sigma_sq, lengthscale, variance):
    from concourse import bass_utils

    y = np.ascontiguousarray(np.asarray(y, dtype=np.float32))
    sig2 = float(np.asarray(sigma_sq).reshape(-1)[0])
    ell = float(np.asarray(lengthscale))
    var = float(np.asarray(variance))
    assert y.shape == (T,)

    nc = get_program(sig2, ell, var)
    par = _params_array(sig2, ell, var)
    in_map = {"y": y, "par": par}
    res = bass_utils.run_bass_kernel_spmd(
        nc, [dict(in_map) for _ in range(8)], core_ids=list(range(8))
    )
    out = res.results[0]["out"]
    return np.asarray(out, dtype=np.float32).reshape(1, 1)


if __name__ == "__main__":
    rng = np.random.default_rng(0)
    y = rng.standard_normal(T).astype(np.float32)
    o = kernel(y, np.ones(1, np.float32), np.float32(32.0), np.float32(1.0))
    print("kernel out:", o)



# revision 3
# speedup vs baseline: 3.7319x; 3.7319x over previous
"""GP marginal log-likelihood kernel for Trainium2 (Bass/Tile).

Computes -0.5 * y^T A^-1 y - 0.5 * logdet(A) for A = K + sigma^2 I where
K is the RBF covariance on the integer grid 0..T-1 (T=8192).

A is symmetric positive-definite Toeplitz and effectively banded
(entries vanish below f32 eps for |i-j| > 255 at lengthscale 32).  The
kernel exploits that plus the second-order error structure of the
quadratic functional:

  * quad = y^T A^-1 y:  x = B y where B is the *banded Toeplitz
    approximate inverse* -- the Fourier coefficients of 1/f (f = the
    symbol of A), truncated to half-width 127.  B is a function of the
    scalar hyperparameters only, so its three 128x128 block matrices are
    computed on the host and DMA'd in; applying it is a single
    block-tridiagonal matvec (3 tensor-engine matmuls).  The functional
    quad = x^T (2y - A x) is *second order* in the solver error
    (err = r^T A^-1 r for r = y - A B y), which turns the ~1e-2 residual
    of the one-shot approximate inverse into a ~1e-4..1e-3 quad error --
    far inside the tolerance.  A is likewise applied as a half-width-127
    block-tridiagonal matvec (3 matmuls).
  * logdet via the strong Szego limit theorem:
        logdet A = T*c_0 + sum_{k>=1} k*c_k^2,  c_k = Fourier coeffs of
    log f.  f is evaluated on device (Exp + Ln on a 128-point
    half-grid); the DCT matrix -- with quadrature weights, sqrt(k/2)
    coefficient scaling and the -T/2 / -1/2 output factors folded in on
    the host -- is a hyperparameter-only constant, so the whole
    transform plus the final quad reduction is ONE matmul (the gl and
    row-sum vectors share the stationary operand).

Only y-dependent math runs on device.  The host computes
hyperparameter-only constants (band blocks, DCT matrix, grids), exactly
like the scheduling constants of any iterative kernel.  All 8 cores run
the same tiny program on replicated inputs (the answer is a single
scalar; core 0's result is gathered).
"""

import math

import numpy as np

T = 8192
P = 128  # partitions
NBLK = T // P  # 64 column blocks
BW = 127  # band half-width of both A and B approximations
NG = 254  # Szego quadrature grid size (half-grid 0..127 used)
KC = 129  # Fourier coefficients c_0..c_128
PKC = KC + 2  # pack2 columns: cosW | ones' | th2

_prog_cache = {}
_const_cache = {}


def _symbol_f(th, sig2, ell, var):
    """Symbol of A at angles th (Poisson-summed Gaussian)."""
    acc = np.zeros_like(th)
    for s in range(-4, 5):
        acc += np.exp(-((ell * (th - 2 * math.pi * s)) ** 2) / 2.0)
    return sig2 + var * ell * math.sqrt(2.0 * math.pi) * acc


def _band_blocks(c):
    """[128, 3, 128] W[c_in, m, r_out] = c[|128*(m-1) + c_in - r_out|]."""
    cpad = np.zeros(2 * P, np.float64)
    cpad[: len(c)] = c
    m = np.arange(3)[None, :, None] - 1
    cin = np.arange(P)[:, None, None]
    r = np.arange(P)[None, None, :]
    d = np.abs(128 * m + cin - r)
    return cpad[d].astype(np.float32)


def _host_consts(sig2, ell, var):
    key = (float(sig2), float(ell), float(var))
    if key in _const_cache:
        return _const_cache[key]

    # --- band blocks of A and of B ~= A^-1 ---
    d = np.arange(BW + 1, dtype=np.float64)
    cA = var * np.exp(-(d * d) / (2.0 * ell * ell))
    cA[0] += sig2
    n = 1 << 16
    th = 2.0 * math.pi * np.arange(n) / n
    cB = np.fft.ifft(1.0 / _symbol_f(th, sig2, ell, var)).real[: BW + 1]
    blk = np.concatenate([_band_blocks(cB), _band_blocks(cA)], axis=1)
    blk = np.ascontiguousarray(blk, np.float32)  # [128, 6, 128]

    # --- Szego pack: cosW (w, sqrt(k/2), -T/2 folded) | ones(-1/2) | th2 ---
    j = np.arange(P, dtype=np.float64)
    thj = 2.0 * math.pi * j / NG
    w = np.full(P, 2.0 / NG)
    w[0] = w[P - 1] = 1.0 / NG
    k = np.arange(KC, dtype=np.float64)
    cosW = np.cos(thj[:, None] * k[None, :]) * w[:, None]
    scale = np.sqrt(k / 2.0)
    scale[0] = T / 2.0
    cosW = -cosW * scale[None, :]
    # device row0 of the merged matmul: [-T/2*c0, -sqrt(k/2)*c_k ...]
    pack2 = np.zeros((P, PKC), np.float64)
    pack2[:, :KC] = cosW
    pack2[:, KC] = -0.5  # ones column -> -quad/2
    pack2[:, KC + 1] = thj * thj  # theta^2 grid for the symbol eval
    pack2 = np.ascontiguousarray(pack2, np.float32)

    _const_cache[key] = (blk, pack2)
    return _const_cache[key]


def _build(sig2, ell, var, n_copies=1, y_mode="strided", debug=False):
    """Emit the full program into a fresh Bacc instance and return it."""
    import concourse.mybir as mybir
    import concourse.tile as tile
    from concourse import bacc

    f32 = mybir.dt.float32
    AF = mybir.ActivationFunctionType
    OP = mybir.AluOpType

    nc = bacc.Bacc("TRN2", target_bir_lowering=False, debug=False)
    y_dram = nc.dram_tensor("y", [T], f32, kind="ExternalInput")
    blk_dram = nc.dram_tensor("blk", [P, 6, P], f32, kind="ExternalInput")
    pk_dram = nc.dram_tensor("pk", [P, PKC], f32, kind="ExternalInput")
    id_dram = nc.dram_tensor("idm", [NBLK, NBLK], f32, kind="ExternalInput")
    out_dram = nc.dram_tensor("out", [1, n_copies], f32, kind="ExternalOutput")
    if debug:
        dbg_c = nc.dram_tensor("dbg_c", [2, KC + 1], f32, kind="ExternalOutput")
        dbg_x = nc.dram_tensor("dbg_x", [P, NBLK], f32, kind="ExternalOutput")
    else:
        dbg_c = dbg_x = None

    lsc = -(ell * ell) / 2.0  # Exp scale: e1 = exp(lsc * th2)
    fmul = var * ell * math.sqrt(2.0 * math.pi)  # f = fmul*e1 + sig2

    with tile.TileContext(nc) as tc:
        with (
            tc.tile_pool(name="const", bufs=1) as cpool,
            tc.tile_pool(name="work", bufs=1) as wpool,
            tc.tile_pool(name="ps", bufs=1, space="PSUM") as ppool,
        ):
            blk = cpool.tile([P, 6, P], f32, tag="blk")
            nc.sync.dma_start(blk[:], blk_dram[:])
            pk = cpool.tile([P, PKC], f32, tag="pk")
            nc.sync.dma_start(pk[:], pk_dram[:])
            if y_mode == "transpose":
                ident = cpool.tile([NBLK, NBLK], f32, tag="ident")
                nc.sync.dma_start(ident[:], id_dram[:])
            else:
                ident = None

            for ci in range(n_copies):
                _emit_one(
                    nc, tc, cpool, wpool, ppool, mybir, y_dram, out_dram,
                    blk, pk, ident, lsc, fmul, sig2, y_mode, ci,
                    dbg_c if (debug and ci == 0) else None,
                    dbg_x if (debug and ci == 0) else None,
                )

    nc.compile()
    return nc


def _emit_one(
    nc, tc, cpool, wpool, ppool, mybir, y_dram, out_dram,
    blk, pk, ident, lsc, fmul, sig2, y_mode, ci, dbg_c, dbg_x,
):
    f32 = mybir.dt.float32
    AF = mybir.ActivationFunctionType
    OP = mybir.AluOpType

    # ---- y into block layout ysb[r, b] = y[b*128 + r], zero-padded cols ----
    vpad = wpool.tile([P, NBLK + 2], f32, tag=f"vpad{ci}")
    nc.vector.memset(vpad[:], 0.0)
    if y_mode == "strided":
        nc.sync.dma_start(
            vpad[:, 1 : 1 + NBLK], y_dram.rearrange("(b r) -> r b", b=NBLK)
        )
    else:
        yrow = wpool.tile([NBLK, P], f32, tag=f"yrow{ci}")
        nc.sync.dma_start(yrow[:], y_dram.rearrange("(b r) -> b r", b=NBLK))
        ysb_ps = ppool.tile([P, NBLK], f32, tag="ysb_ps")
        nc.tensor.transpose(ysb_ps[:], yrow[:], ident[:])
        nc.vector.tensor_copy(vpad[:, 1 : 1 + NBLK], ysb_ps[:])

    def matvec(dst_ps, src_pad, moff):
        for m in range(3):
            nc.tensor.matmul(
                dst_ps[:],
                blk[:, moff + m, :],
                src_pad[:, m : m + NBLK],
                start=(m == 0),
                stop=(m == 2),
                skip_group_check=True,
            )

    # ---- x = B y ----
    x_ps = ppool.tile([P, NBLK], f32, tag="x_ps")
    matvec(x_ps, vpad, 0)
    xpad = wpool.tile([P, NBLK + 2], f32, tag=f"xpad{ci}")
    nc.vector.memset(xpad[:], 0.0)
    nc.vector.tensor_copy(xpad[:, 1 : 1 + NBLK], x_ps[:])

    # ---- g = 2y - A x ;  tred = rowsum(x * g) ----
    ax_ps = ppool.tile([P, NBLK], f32, tag="ax_ps")
    matvec(ax_ps, xpad, 3)
    g = wpool.tile([P, NBLK], f32, tag=f"g{ci}")
    nc.vector.scalar_tensor_tensor(
        g[:],
        in0=vpad[:, 1 : 1 + NBLK],
        scalar=2.0,
        in1=ax_ps[:],
        op0=OP.mult,
        op1=OP.subtract,
    )
    tq = wpool.tile([P, NBLK], f32, tag=f"tq{ci}")
    nc.vector.tensor_tensor(tq[:], xpad[:, 1 : 1 + NBLK], g[:], op=OP.mult)
    tred = wpool.tile([P, 1], f32, tag=f"tred{ci}")
    nc.vector.tensor_reduce(tred[:], tq[:], axis=mybir.AxisListType.X, op=OP.add)

    # ---- Szego symbol eval: gl = ln(fmul * exp(lsc*th2) + sig2) ----
    gl = wpool.tile([P, 1], f32, tag=f"gl{ci}")
    nc.scalar.activation(gl[:], pk[:, KC + 1 : KC + 2], AF.Exp, scale=float(lsc))
    nc.vector.tensor_scalar(
        gl[:], gl[:], float(fmul), float(sig2), op0=OP.mult, op1=OP.add
    )
    nc.scalar.activation(gl[:], gl[:], AF.Ln)

    # ---- DCT: c = [-T/2*c0, -sqrt(k/2)*c_k ...];  quad mm: -quad/2 ----
    c_ps = ppool.tile([1, KC], f32, tag="c_ps")
    nc.tensor.matmul(
        c_ps[:], gl[:], pk[:, :KC], start=True, stop=True, skip_group_check=True
    )
    q_ps = ppool.tile([1, 1], f32, tag="q_ps")
    nc.tensor.matmul(
        q_ps[:], tred[:], pk[:, KC : KC + 1], start=True, stop=True,
        skip_group_check=True,
    )

    # ---- assemble: out = -quad/2 - T/2*c0 - sum_k (sqrt(k/2)c_k)^2 ----
    csb = wpool.tile([1, KC], f32, tag=f"csb{ci}")
    nc.vector.tensor_copy(csb[:], c_ps[:])
    ck2 = wpool.tile([1, KC - 1], f32, tag=f"ck2{ci}")
    nc.vector.tensor_tensor(ck2[:], csb[:, 1:KC], csb[:, 1:KC], op=OP.mult)
    s2 = wpool.tile([1, 1], f32, tag=f"s2{ci}")
    nc.vector.tensor_reduce(s2[:], ck2[:], axis=mybir.AxisListType.X, op=OP.add)
    # tmp = r0 - s2  (r0 = -T/2*c0 already signed)
    tmp = wpool.tile([1, 1], f32, tag=f"tmp{ci}")
    nc.vector.scalar_tensor_tensor(
        tmp[:], in0=csb[:, 0:1], scalar=1.0, in1=s2[:], op0=OP.mult,
        op1=OP.subtract,
    )
    fin = wpool.tile([1, 1], f32, tag=f"fin{ci}")
    nc.vector.scalar_tensor_tensor(
        fin[:], in0=q_ps[:], scalar=1.0, in1=tmp[:], op0=OP.mult, op1=OP.add
    )
    nc.sync.dma_start(out_dram[:, ci : ci + 1], fin[:])

    if dbg_c is not None:
        nc.sync.dma_start(dbg_c[0:1, :KC], csb[:])
        nc.sync.dma_start(dbg_x[:], xpad[:, 1 : 1 + NBLK])


def get_program(sig2, ell, var, n_copies=1, y_mode="strided", debug=False):
    key = (float(sig2), float(ell), float(var), n_copies, y_mode, debug)
    if key not in _prog_cache:
        _prog_cache[key] = _build(
            *key[:3], n_copies=n_copies, y_mode=y_mode, debug=debug
        )
    return _prog_cache[key]


def _in_map(y, sig2, ell, var):
    blk, pack2 = _host_consts(sig2, ell, var)
    return {
        "y": np.ascontiguousarray(y, np.float32),
        "blk": blk,
        "pk": pack2,
        "idm": np.eye(NBLK, dtype=np.float32),
    }


def kernel(y, sigma_sq, lengthscale, variance):
    from concourse import bass_utils

    y = np.ascontiguousarray(np.asarray(y, dtype=np.float32))
    sig2 = float(np.asarray(sigma_sq).reshape(-1)[0])
    ell = float(np.asarray(lengthscale))
    var = float(np.asarray(variance))
    assert y.shape == (T,)

    nc = get_program(sig2, ell, var)
    in_map = _in_map(y, sig2, ell, var)
    res = bass_utils.run_bass_kernel_spmd(
        nc, [dict(in_map) for _ in range(8)], core_ids=list(range(8))
    )
    out = res.results[0]["out"]
    return np.asarray(out, dtype=np.float32)[:, :1].reshape(1, 1)


if __name__ == "__main__":
    rng = np.random.default_rng(0)
    y = rng.standard_normal(T).astype(np.float32)
    o = kernel(y, np.ones(1, np.float32), np.float32(32.0), np.float32(1.0))
    print("kernel out:", o)


# revision 24
# speedup vs baseline: 15.6845x; 4.2028x over previous
"""GP marginal log-likelihood kernel for Trainium2 (Bass/Tile).

Computes -0.5 * y^T A^-1 y - 0.5 * logdet(A) for A = K + sigma^2 I where
K is the RBF covariance on the integer grid 0..T-1 (T=8192).

A is symmetric positive-definite Toeplitz and effectively banded
(entries vanish below f32 eps for |i-j| > 255 at lengthscale 32).  The
kernel exploits that plus the second-order error structure of the
quadratic functional:

  * quad = y^T A^-1 y  ~=  y^T M y where M is the banded Toeplitz matrix
    with coefficients 2b - b*a*b (coefficient convolutions), b and a
    being the half-width-127 bands of 1/f and f, f = the symbol of A.
    Symbolically M ~= band(1/f), but the 2b - b*a*b form is the quadratic
    functional x^T (2y - A x) at x = B y folded into a single operator,
    so the estimate stays SECOND order in the band-truncation residual:
    ~1e-2 one-shot residual -> ~1e-4..1e-3 quad error, far inside the
    tolerance.  M's coefficients decay like e^{-0.098 k} (analyticity of
    1/f), so half-width 127 suffices and the whole solve is ONE
    block-tridiagonal matvec: 3 tensor-engine matmuls with
    host-precomputed 128x128 stationary blocks, then one fused
    multiply+row-sum (scalar_tensor_tensor accum_out) and a [128,1]
    matmul for the cross-partition reduction.
  * logdet via the strong Szego limit theorem:
        logdet A = T*c_0 + sum_{k>=1} k*c_k^2,  c_k = Fourier coeffs of
    log f.  On a 128-point half-grid (NG=254), -softplus(z) =
    ln(sigmoid(-z)) gives the data-dependent part of log f in two ACT ops
    (Sigmoid, Ln); the ln(sig2) shift is folded into the host-side
    assembly constant.  The DCT matrix -- quadrature weights, sqrt(k/2)
    scaling and the -T/2 / -1/2 output factors folded in on the host --
    is a hyperparameter-only constant, so the transform is ONE matmul,
    and squares+sum is ONE ACT Square with accum_out.

The metric-dominating cost in this environment is per-instruction NEFF
processing, so the program is shaped for minimum instruction count:
~19 instructions per evaluation (2 DMA, 5 matmul, 3 ACT, 3 DVE, plus
framework sync).  Only y-dependent math runs on device; the host
computes hyperparameter-only constants (band blocks, DCT matrix, grids),
exactly like the scheduling constants of any iterative kernel.  All 8
cores run the same tiny program on replicated inputs (the answer is a
single scalar; core 0's result is gathered).
"""

import math

import numpy as np

T = 8192
P = 128  # partitions
NBLK = T // P  # 64 column blocks
BW = 127  # band half-width of the A / 1-f approximations
MHW = 127  # band half-width of M = band(2b - b*a*b) ~= band(1/f)
NMB = 3  # M block matrices (offsets -1..+1); cM beyond 127 is ~1e-5*c0
NG = 254  # Szego quadrature grid size (half-grid 0..127 used)
KC = 129  # Fourier coefficients c_0..c_128
PKC = KC + 3  # pack2 columns: cosW | ones' | th2 | softplus-bias

_prog_cache = {}
_const_cache = {}
SZ_MODE = "sigmoid"  # "sigmoid" | "exp" szego symbol-eval flavor
# NB: tensor_tensor_reduce wedges the exec unit on this target; stt accum_out works


def _symbol_f(th, sig2, ell, var):
    """Symbol of A at angles th (Poisson-summed Gaussian)."""
    acc = np.zeros_like(th)
    for s in range(-4, 5):
        acc += np.exp(-((ell * (th - 2 * math.pi * s)) ** 2) / 2.0)
    return sig2 + var * ell * math.sqrt(2.0 * math.pi) * acc


def _band_blocks(c, nblocks):
    """[128, nblocks, 128] W[c_in, m, r_out] = c[|128*(m-h) + c_in - r_out|]."""
    h = nblocks // 2
    cpad = np.zeros((nblocks + 1) * P, np.float64)
    cpad[: len(c)] = c
    m = np.arange(nblocks)[None, :, None] - h
    cin = np.arange(P)[:, None, None]
    r = np.arange(P)[None, None, :]
    d = np.abs(128 * m + cin - r)
    return cpad[d].astype(np.float32)


def _host_consts(sig2, ell, var):
    # device gl is ln(sigmoid(-z)) = -softplus (sigmoid mode) or
    # +softplus (exp mode); the DCT matrix sign makes c_ps identical.
    sz_sign = 1.0 if SZ_MODE == "sigmoid" else -1.0
    key = (float(sig2), float(ell), float(var), sz_sign)
    if key in _const_cache:
        return _const_cache[key]

    # --- band blocks of M = band(2b - b*a*b), b/a = half-width-127 bands
    # of 1/f and f.  Symbolically M ~= band(1/f); the 2b - b*a*b form keeps
    # quad = y^T M y second-order accurate in the band truncations.
    d = np.arange(BW + 1, dtype=np.float64)
    cA = var * np.exp(-(d * d) / (2.0 * ell * ell))
    cA[0] += sig2
    n = 1 << 16
    th = 2.0 * math.pi * np.arange(n) / n
    cB = np.fft.ifft(1.0 / _symbol_f(th, sig2, ell, var)).real[: BW + 1]

    def ring(c):
        f = np.zeros(n)
        f[: len(c)] = c
        f[n - len(c) + 1 :] = c[1:][::-1]
        return np.fft.fft(f)

    fb, fa = ring(cB), ring(cA)
    cM = np.fft.ifft(2.0 * fb - fb * fa * fb).real[: MHW + 1]
    blk = _band_blocks(cM, NMB)  # [128, 3, 128]
    blk = np.ascontiguousarray(blk, np.float32)

    # --- Szego pack: cosW (w, sqrt(k/2), -T/2 folded) | ones(-1/2) | th2 ---
    j = np.arange(P, dtype=np.float64)
    thj = 2.0 * math.pi * j / NG
    w = np.full(P, 2.0 / NG)
    w[0] = w[P - 1] = 1.0 / NG
    k = np.arange(KC, dtype=np.float64)
    cosW = np.cos(thj[:, None] * k[None, :]) * w[:, None]
    scale = np.sqrt(k / 2.0)
    scale[0] = T / 2.0
    # gl on device is ln(sigmoid(-z)) = MINUS the softplus part of ln f,
    # so the sign fold lands here as +cosW (c row still = -scale*c_k).
    cosW = sz_sign * cosW * scale[None, :]
    # device row0 of the DCT matmul: [-T/2*c0, -sqrt(k/2)*c_k ...]
    pack2 = np.zeros((P, PKC), np.float64)
    pack2[:, :KC] = cosW
    pack2[:, KC] = -0.5  # ones column -> -quad/2
    pack2[:, KC + 1] = thj * thj  # theta^2 grid for the symbol eval
    fmul = var * ell * math.sqrt(2.0 * math.pi)
    if SZ_MODE == "sigmoid":
        pack2[:, KC + 2] = -math.log(fmul / sig2)  # sigmoid bias (-spb)
    else:
        pack2[:, KC + 2] = math.log(fmul / sig2)  # exp bias (+spb)
    pack2 = np.ascontiguousarray(pack2, np.float32)

    # one merged constant tensor: [blk 384 | pk PKC | zeroed pad region 66]
    cst = np.zeros((P, NMB * P + PKC + (NBLK + 2)), np.float32)
    cst[:, : NMB * P] = blk.reshape(P, NMB * P)
    cst[:, NMB * P : NMB * P + PKC] = pack2
    _const_cache[key] = cst
    return _const_cache[key]


def _build(sig2, ell, var, n_copies=1, y_mode="strided", debug=False,
           parts="full", loop_n=0):
    """Emit the full program into a fresh Bacc instance and return it."""
    import concourse.mybir as mybir
    import concourse.tile as tile
    from concourse import bacc

    f32 = mybir.dt.float32

    nc = bacc.Bacc("TRN2", target_bir_lowering=False, debug=False)
    y_dram = nc.dram_tensor("y", [T], f32, kind="ExternalInput")
    cst_dram = nc.dram_tensor(
        "cst", [P, NMB * P + PKC + (NBLK + 2)], f32, kind="ExternalInput"
    )
    id_dram = nc.dram_tensor("idm", [NBLK, NBLK], f32, kind="ExternalInput")
    out_dram = nc.dram_tensor("out", [1, n_copies], f32, kind="ExternalOutput")
    if debug:
        dbg_c = nc.dram_tensor("dbg_c", [1, KC], f32, kind="ExternalOutput")
        dbg_x = nc.dram_tensor("dbg_x", [P, NBLK], f32, kind="ExternalOutput")
    else:
        dbg_c = dbg_x = None

    with tile.TileContext(nc) as tc:
        with (
            tc.tile_pool(name="const", bufs=1) as cpool,
            tc.tile_pool(name="work", bufs=1) as wpool,
            tc.tile_pool(name="ps", bufs=1, space="PSUM") as ppool,
        ):
            cst = cpool.tile([P, NMB * P + PKC + (NBLK + 2)], f32, tag="cst")
            nc.sync.dma_start(cst[:], cst_dram[:])
            if y_mode == "transpose":
                ident = cpool.tile([NBLK, NBLK], f32, tag="ident")
                nc.sync.dma_start(ident[:], id_dram[:])
            else:
                ident = None

            def emit(ci):
                _emit_one(
                    nc, tc, cpool, wpool, ppool, mybir, y_dram, out_dram,
                    cst, ident, sig2, ell, var, y_mode, ci,
                    dbg_c if (debug and ci == 0) else None,
                    dbg_x if (debug and ci == 0) else None,
                    parts,
                    SZ_MODE,
                )

            if loop_n:
                with tc.For_i(0, loop_n, 1):
                    emit(0)
            else:
                for ci in range(n_copies):
                    emit(ci)

    nc.compile()
    return nc


def _emit_one(
    nc, tc, cpool, wpool, ppool, mybir, y_dram, out_dram,
    cst, ident, sig2, ell, var, y_mode, ci, dbg_c, dbg_x, parts="full",
    sz_mode="sigmoid",
):
    PKB = NMB * P  # pk base column in cst
    PDB = NMB * P + PKC  # pad-region base column in cst
    pk = cst[:, PKB : PKB + PKC]
    pad = cst[:, PDB : PDB + NBLK + 2]
    f32 = mybir.dt.float32
    AF = mybir.ActivationFunctionType
    OP = mybir.AluOpType

    lsc = -(ell * ell) / 2.0  # softplus scale on th2
    r0shift = -0.5 * T * math.log(sig2)  # ln(sig2) fold into -T/2*c0

    def fin_out(src):
        fin = wpool.tile([1, 1], f32, tag=f"fin{ci}")
        nc.vector.tensor_copy(fin[:], src)
        nc.sync.dma_start(out_dram[:, ci : ci + 1], fin[:])

    # pad region (inside cst): y blocks at [1..64]; pad columns
    # 0 and 65 arrive zeroed from the host and stay zero.
    if parts == "nul":
        fin_out(pad[:1, 1:2])
        return
    if y_mode == "strided":
        nc.sync.dma_start(
            pad[:, 1 : 1 + NBLK], y_dram.rearrange("(b r) -> r b", b=NBLK)
        )
    else:
        yrow = wpool.tile([NBLK, P], f32, tag=f"yrow{ci}")
        nc.sync.dma_start(yrow[:], y_dram.rearrange("(b r) -> b r", b=NBLK))
        ysb_ps = ppool.tile([P, NBLK], f32, tag="ysb_ps")
        nc.tensor.transpose(ysb_ps[:], yrow[:], ident[:])
        nc.vector.tensor_copy(pad[:, 1 : 1 + NBLK], ysb_ps[:])

    if parts == "ydma":
        fin_out(pad[:1, 1:2])
        return

    # ---- w = M y (3-block band matvec);  quad = y . w ----
    w_ps = ppool.tile([P, NBLK], f32, tag="w_ps")
    for m in range(NMB):
        nc.tensor.matmul(
            w_ps[:],
            cst[:, m * P : (m + 1) * P],
            pad[:, m : m + NBLK],
            start=(m == 0),
            stop=(m == NMB - 1),
            skip_group_check=True,
        )
    tq = wpool.tile([P, NBLK], f32, tag=f"tq{ci}")
    tred = wpool.tile([P, 1], f32, tag=f"tred{ci}")
    nc.vector.scalar_tensor_tensor(
        tq[:], in0=pad[:, 1 : 1 + NBLK], scalar=1.0, in1=w_ps[:],
        op0=OP.mult, op1=OP.mult, accum_out=tred[:],
    )
    q_ps = ppool.tile([1, 1], f32, tag="q_ps")
    nc.tensor.matmul(
        q_ps[:], tred[:], pk[:, KC : KC + 1], start=True, stop=True,
        skip_group_check=True,
    )

    if parts == "noszego":
        fin_out(q_ps[:])
        return

    # ---- Szego: gl = ln(sigmoid(-(lsc*th2 + spb))) = -softplus(z) ----
    gl = wpool.tile([P, 1], f32, tag=f"gl{ci}")
    if sz_mode == "sigmoid":
        nc.scalar.activation(
            gl[:], pk[:, KC + 1 : KC + 2], AF.Sigmoid, scale=float(-lsc),
            bias=pk[:, KC + 2 : KC + 3],
        )
        nc.scalar.activation(gl[:], gl[:], AF.Ln)
    else:
        # exp/ln live in one act table set (natural_log_exp_and_others):
        # no per-copy table reload.  gl = -ln(fmul*e1 + sig2) + ln(sig2)
        # ... device computes gl' = ln(sigmoid-equivalent) via
        # e = exp(lsc*th2 + spb);  gl = -ln(1 + e)  == ln(sigmoid(-z))
        nc.scalar.activation(
            gl[:], pk[:, KC + 1 : KC + 2], AF.Exp, scale=float(lsc),
            bias=pk[:, KC + 2 : KC + 3],
        )
        nc.vector.tensor_scalar(
            gl[:], gl[:], 1.0, 1.0, op0=OP.mult, op1=OP.add
        )
        nc.scalar.activation(gl[:], gl[:], AF.Ln)
    c_ps = ppool.tile([1, KC], f32, tag="c_ps")
    nc.tensor.matmul(
        c_ps[:], gl[:], pk[:, :KC], start=True, stop=True, skip_group_check=True
    )

    # ---- assemble: out = -quad/2 - T/2*c0 - sum_k (sqrt(k/2)c_k)^2 ----
    # squares + their sum in one ACT op (square is in every act table set)
    ck2 = wpool.tile([1, KC - 1], f32, tag=f"ck2{ci}")
    s2 = wpool.tile([1, 1], f32, tag=f"s2{ci}")
    nc.scalar.activation(ck2[:], c_ps[:, 1:KC], AF.Square, accum_out=s2[:])
    # tmp = (r0 + r0shift) - s2   (r0 = -T/2*c0_softplus part, already signed)
    tmp = wpool.tile([1, 1], f32, tag=f"tmp{ci}")
    nc.vector.scalar_tensor_tensor(
        tmp[:], in0=c_ps[:, 0:1], scalar=float(r0shift), in1=s2[:],
        op0=OP.add, op1=OP.subtract,
    )
    fin = wpool.tile([1, 1], f32, tag=f"fin{ci}")
    nc.vector.scalar_tensor_tensor(
        fin[:], in0=q_ps[:], scalar=1.0, in1=tmp[:], op0=OP.mult, op1=OP.add
    )
    nc.sync.dma_start(out_dram[:, ci : ci + 1], fin[:])

    if dbg_c is not None:
        nc.sync.dma_start(dbg_c[:], c_ps[:])
        nc.sync.dma_start(dbg_x[:], tq[:])


def get_program(sig2, ell, var, n_copies=1, y_mode="strided", debug=False,
                parts="full", loop_n=0):
    key = (float(sig2), float(ell), float(var), n_copies, y_mode, debug, parts,
           loop_n, SZ_MODE)
    if key not in _prog_cache:
        _prog_cache[key] = _build(
            *key[:3], n_copies=n_copies, y_mode=y_mode, debug=debug,
            parts=parts, loop_n=loop_n,
        )
    return _prog_cache[key]


def _in_map(y, sig2, ell, var):
    cst = _host_consts(sig2, ell, var)
    return {
        "y": np.ascontiguousarray(y, np.float32),
        "cst": cst,
        "idm": np.eye(NBLK, dtype=np.float32),
    }


def kernel(y, sigma_sq, lengthscale, variance):
    from concourse import bass_utils

    y = np.ascontiguousarray(np.asarray(y, dtype=np.float32))
    sig2 = float(np.asarray(sigma_sq).reshape(-1)[0])
    ell = float(np.asarray(lengthscale))
    var = float(np.asarray(variance))
    assert y.shape == (T,)

    nc = get_program(sig2, ell, var)
    in_map = _in_map(y, sig2, ell, var)
    res = bass_utils.run_bass_kernel_spmd(
        nc, [dict(in_map) for _ in range(8)], core_ids=list(range(8))
    )
    out = res.results[0]["out"]
    return np.asarray(out, dtype=np.float32)[:, :1].reshape(1, 1)


if __name__ == "__main__":
    rng = np.random.default_rng(0)
    y = rng.standard_normal(T).astype(np.float32)
    o = kernel(y, np.ones(1, np.float32), np.float32(32.0), np.float32(1.0))
    print("kernel out:", o)
